# revision 14
# baseline (speedup 1.0000x reference)
"""ODE-RNN encoder (GRU-ODE scan) Trainium2 Bass kernel, v2 (bf16).

Strategy (data-parallel over trajectories):
  - 4096 trajectories sharded 512/core over 8 NeuronCores; weights
    replicated; the T=128 time scan runs locally per core. Host gathers
    per-core z0 outputs.
  - Feature-on-partition, batch-on-free-dim layout. Each core's 512-batch
    is split into 2 dephased chunks of 256 emitted half a step apart
    (anti-phase ordering edges) so the two independent serial chains
    interleave in every engine queue.
  - All matmul operands are bf16 (PSUM accumulation stays fp32): on HW the
    PE streams bf16 at ~2x the fp32r rate. Gates are computed from the
    post-Euler state directly (no fused-ODE-correction matmuls), giving 11
    matmuls per chunk-step. Activations are merged: one tanh over the
    [u|r] gate-1 PSUM block and one sigmoid over the [v|r] gate-2 block
    (4 Act instructions per chunk-step). The u-gate weights are negated +
    duplicated so sigmoid directly yields v = 1-u on 128 partitions; r is
    duplicated likewise for the one-shot r*[y;s] multiply.
  - The mask-blend runs on DVE in bf16 (2-byte SBUF ops hit the 2x/4x DVE
    modes): diff_top = ns_y - y_ode, diff_bot = abs_max(ns_s,0) - s (one
    fused scalar_tensor_tensor), gtq = (m*v) (.) diff, S' = S + gtq.
  - bf16 end-to-end numerics validated against the reference in numpy:
    rel err ~6e-3 (tolerance 2e-2).

kernel(**inputs) takes the full unsharded numpy inputs and returns
(z0_mu, z0_std), each (1, 4096, 64) float32.
"""

import os
import sys

import numpy as np

N_TRAJ = 4096
T = 128
LAT = 64
NDATA = 64
INP = 2 * NDATA
NGRU = 100
NODE = 100
TZ = 100
NCORES = 8
B = N_TRAJ // NCORES          # 512 per core
CH = 2                        # chunks per core
BC = B // CH                  # 256 batch per chunk

_cache = {}


def _build(dts, use_bias):
    import concourse.bass as bass
    import concourse.tile as tile
    from concourse import bacc, mybir

    f32 = mybir.dt.float32
    bf16 = mybir.dt.bfloat16
    ACT = mybir.ActivationFunctionType
    ALU = mybir.AluOpType

    nc = bacc.Bacc("TRN2", target_bir_lowering=False, debug=False,
                   num_devices=NCORES)

    # ---- DRAM I/O ----
    xT_d = nc.dram_tensor("xT", [T, INP, B], bf16, kind="ExternalInput")
    wug1_d = nc.dram_tensor("wug1", [2 * LAT + INP, NGRU], bf16, kind="ExternalInput")
    wrg1_d = nc.dram_tensor("wrg1", [2 * LAT + INP, NGRU], bf16, kind="ExternalInput")
    wns1_d = nc.dram_tensor("wns1", [2 * LAT + INP, NGRU], bf16, kind="ExternalInput")
    wug2_d = nc.dram_tensor("wug2nd", [NGRU, 2 * LAT], bf16, kind="ExternalInput")
    wrg2_d = nc.dram_tensor("wrg2d", [NGRU, 2 * LAT], bf16, kind="ExternalInput")
    wns2_d = nc.dram_tensor("wns2", [NGRU, 2 * LAT], bf16, kind="ExternalInput")
    wode1_d = nc.dram_tensor("wode1", [LAT, NODE], bf16, kind="ExternalInput")
    wode2_d = nc.dram_tensor("wode2", [NODE, LAT], bf16, kind="ExternalInput")
    wtz1_d = nc.dram_tensor("wtz1", [2 * LAT, TZ], bf16, kind="ExternalInput")
    wtz2_d = nc.dram_tensor("wtz2", [TZ, 2 * LAT], bf16, kind="ExternalInput")
    if use_bias:
        bode1_d = nc.dram_tensor("bode1", [NODE, 1], f32, kind="ExternalInput")
        bns1_d = nc.dram_tensor("bns1", [NGRU, 1], f32, kind="ExternalInput")
        btz1_d = nc.dram_tensor("btz1", [TZ, 1], f32, kind="ExternalInput")
        btz2t_d = nc.dram_tensor("btz2t", [LAT, 1], f32, kind="ExternalInput")
        btz2b_d = nc.dram_tensor("btz2b", [LAT, 1], f32, kind="ExternalInput")
        # row-vector biases (K=1 matmul accumulate): [1, M]
        bug1_d = nc.dram_tensor("bug1r", [1, NGRU], bf16, kind="ExternalInput")
        brg1_d = nc.dram_tensor("brg1r", [1, NGRU], bf16, kind="ExternalInput")
        bug2_d = nc.dram_tensor("bug2ndr", [1, 2 * LAT], bf16, kind="ExternalInput")
        brg2_d = nc.dram_tensor("brg2dr", [1, 2 * LAT], bf16, kind="ExternalInput")
        bns2_d = nc.dram_tensor("bns2r", [1, 2 * LAT], bf16, kind="ExternalInput")
        bode2_d = nc.dram_tensor("bode2r", [1, LAT], bf16, kind="ExternalInput")
        ones_d = nc.dram_tensor("ones1", [1, BC], bf16, kind="ExternalInput")
    zeros_d = nc.dram_tensor("zeros0", [2 * LAT, B], bf16, kind="ExternalInput")
    zout_d = nc.dram_tensor("zout", [2 * LAT, B], f32, kind="ExternalOutput")

    with tile.TileContext(nc) as tc:
        with (
            tc.tile_pool(name="const", bufs=1) as cpool,
            tc.tile_pool(name="state", bufs=1) as spool,
            tc.tile_pool(name="xin", bufs=3) as xpool,
            tc.tile_pool(name="mdup", bufs=2) as mpool,
            tc.tile_pool(name="tmp0", bufs=2) as tpool0,
            tc.tile_pool(name="tmp1", bufs=2) as tpool1,
            tc.tile_pool(name="psA0", bufs=1, space="PSUM") as psA0,
            tc.tile_pool(name="psB0", bufs=1, space="PSUM") as psB0,
            tc.tile_pool(name="g1p0", bufs=1, space="PSUM") as g1p0,
            tc.tile_pool(name="g2p0", bufs=1, space="PSUM") as g2p0,
            tc.tile_pool(name="psA1", bufs=1, space="PSUM") as psA1,
            tc.tile_pool(name="psB1", bufs=1, space="PSUM") as psB1,
            tc.tile_pool(name="g1p1", bufs=1, space="PSUM") as g1p1,
            tc.tile_pool(name="g2p1", bufs=1, space="PSUM") as g2p1,
        ):
            tpool = [tpool0, tpool1]
            psA = [psA0, psA1]
            psB = [psB0, psB1]
            g1p = [g1p0, g1p1]
            g2p = [g2p0, g2p1]

            # ---- load constants ----
            def cload(shape, src_ap, tag, dt_=bf16):
                t = cpool.tile(shape, dt_, tag=tag, name=tag)
                nc.sync.dma_start(t[:, :], src_ap)
                return t

            wug1a = cload([2 * LAT, NGRU], wug1_d[0:2 * LAT, :], "wug1a")
            wug1b = cload([INP, NGRU], wug1_d[2 * LAT:2 * LAT + INP, :], "wug1b")
            wrg1a = cload([2 * LAT, NGRU], wrg1_d[0:2 * LAT, :], "wrg1a")
            wrg1b = cload([INP, NGRU], wrg1_d[2 * LAT:2 * LAT + INP, :], "wrg1b")
            wns1a = cload([2 * LAT, NGRU], wns1_d[0:2 * LAT, :], "wns1a")
            wns1b = cload([INP, NGRU], wns1_d[2 * LAT:2 * LAT + INP, :], "wns1b")
            wug2 = cload([NGRU, 2 * LAT], wug2_d[:, :], "wug2")
            wrg2 = cload([NGRU, 2 * LAT], wrg2_d[:, :], "wrg2")
            wns2 = cload([NGRU, 2 * LAT], wns2_d[:, :], "wns2")
            wode1 = cload([LAT, NODE], wode1_d[:, :], "wode1")
            wode2 = cload([NODE, LAT], wode2_d[:, :], "wode2")
            wtz1 = cload([2 * LAT, TZ], wtz1_d[:, :], "wtz1")
            wtz2 = cload([TZ, 2 * LAT], wtz2_d[:, :], "wtz2")
            if use_bias:
                bode1 = cload([NODE, 1], bode1_d[:, :], "bode1", f32)
                bns1 = cload([NGRU, 1], bns1_d[:, :], "bns1", f32)
                btz1 = cload([TZ, 1], btz1_d[:, :], "btz1", f32)
                btz2t = cload([LAT, 1], btz2t_d[:, :], "btz2t", f32)
                btz2b = cpool.tile([2 * LAT, 1], f32, tag="btz2b", name="btz2b")
                nc.sync.dma_start(btz2b[LAT:2 * LAT, :], btz2b_d[:, :])
                bug1r = cload([1, NGRU], bug1_d[:, :], "bug1r")
                brg1r = cload([1, NGRU], brg1_d[:, :], "brg1r")
                bug2r = cload([1, 2 * LAT], bug2_d[:, :], "bug2r")
                brg2r = cload([1, 2 * LAT], brg2_d[:, :], "brg2r")
                bns2r = cload([1, 2 * LAT], bns2_d[:, :], "bns2r")
                bode2r = cload([1, LAT], bode2_d[:, :], "bode2r")
                ones = cload([1, BC], ones_d[:, :], "ones")

            # ---- state tiles (ping-pong per chunk) ----
            S = [[spool.tile([2 * LAT, BC], bf16, tag=f"s{c}_{p}",
                             name=f"s{c}_{p}")
                  for p in range(2)] for c in range(CH)]
            for c in range(CH):
                nc.sync.dma_start(S[c][0][:, :],
                                  zeros_d[:, c * BC:(c + 1) * BC])

            # ---- the scan ----
            def new_ctx(c, t):
                return dict(cs=slice(c * BC, (c + 1) * BC),
                            Sc=S[c][t % 2], Sn=S[c][(t + 1) % 2],
                            tp=tpool[c], t=t)

            # -- PE stages --
            def s_ode1(c, d, xt, m2):
                d['xt'], d['m2'] = xt, m2
                d['ps_oh'] = psA[c].tile([NODE, BC], f32, tag="psA",
                                         name=f"oh{c}")
                # ode hidden bias (bode1) is applied via the Act bias input
                d['ode1'] = nc.tensor.matmul(d['ps_oh'][:, :], wode1[:, :],
                                             d['Sc'][0:LAT, :],
                                             start=True, stop=True)

            def s_xu(c, d, xt, m2):
                d['g1'] = g1p[c].tile([NGRU, 2 * BC], f32, tag="g1",
                                      name=f"g1_{c}")
                g1 = d['g1']
                nc.tensor.matmul(g1[:, 0:BC], wug1b[:, :],
                                 xt[:, d['cs']], start=True, stop=False)
                if use_bias:
                    nc.tensor.matmul(g1[:, 0:BC], bug1r[:, :],
                                     ones[:, :], start=False, stop=False)

            def s_xr(c, d, xt, m2):
                g1 = d['g1']
                nc.tensor.matmul(g1[:, BC:2 * BC], wrg1b[:, :],
                                 xt[:, d['cs']], start=True, stop=False)
                if use_bias:
                    nc.tensor.matmul(g1[:, BC:2 * BC], brg1r[:, :],
                                     ones[:, :], start=False, stop=False)

            def s_ode2(c, d, xt, m2):
                d['ps_yo'] = psB[c].tile([LAT, BC], f32, tag="psB",
                                         name=f"yo{c}")
                nc.tensor.matmul(d['ps_yo'][:, :], wode2[:, :],
                                 d['h_ode'][:, :], start=True,
                                 stop=not use_bias)
                if use_bias:
                    nc.tensor.matmul(d['ps_yo'][:, :], bode2r[:, :],
                                     ones[:, :], start=False, stop=True)

            def s_xn(c, d, xt, m2):
                d['n1'] = psA[c].tile([NGRU, BC], f32, tag="psA",
                                      name=f"n1_{c}")
                nc.tensor.matmul(d['n1'][:, :], wns1b[:, :],
                                 xt[:, d['cs']], start=True, stop=False)

            def s_su(c, d, xt, m2):
                nc.tensor.matmul(d['g1'][:, 0:BC], wug1a[:, :],
                                 d['Sc'][:, :], start=False, stop=True)

            def s_sr(c, d, xt, m2):
                nc.tensor.matmul(d['g1'][:, BC:2 * BC], wrg1a[:, :],
                                 d['Sc'][:, :], start=False, stop=True)

            def s_g2u(c, d, xt, m2):
                d['g2'] = g2p[c].tile([2 * LAT, 2 * BC], f32, tag="g2",
                                      name=f"g2_{c}")
                nc.tensor.matmul(d['g2'][:, 0:BC], wug2[:, :],
                                 d['h_g'][:, 0:BC],
                                 start=True, stop=not use_bias)
                if use_bias:
                    nc.tensor.matmul(d['g2'][:, 0:BC], bug2r[:, :],
                                     ones[:, :], start=False, stop=True)

            def s_g2r(c, d, xt, m2):
                nc.tensor.matmul(d['g2'][:, BC:2 * BC], wrg2[:, :],
                                 d['h_g'][:, BC:2 * BC],
                                 start=True, stop=not use_bias)
                if use_bias:
                    nc.tensor.matmul(d['g2'][:, BC:2 * BC], brg2r[:, :],
                                     ones[:, :], start=False, stop=True)

            def s_ns1(c, d, xt, m2):
                nc.tensor.matmul(d['n1'][:, :], wns1a[:, :],
                                 d['ryc'][:, :], start=False, stop=True)

            def s_ns2(c, d, xt, m2):
                d['n2'] = psB[c].tile([2 * LAT, BC], f32, tag="psB",
                                      name=f"n2_{c}")
                nc.tensor.matmul(d['n2'][:, :], wns2[:, :],
                                 d['h_n'][:, :], start=True,
                                 stop=not use_bias)
                if use_bias:
                    nc.tensor.matmul(d['n2'][:, :], bns2r[:, :],
                                     ones[:, :], start=False, stop=True)

            # -- Act stages --
            def s_tanh_ode(c, d, xt, m2):
                d['h_ode'] = d['tp'].tile([NODE, BC], bf16, tag="h_ode",
                                          name=f"ho{c}")
                nc.scalar.activation(d['h_ode'][:, :], d['ps_oh'][:, :],
                                     ACT.Tanh,
                                     bias=bode1[:, :] if use_bias else 0.0)

            def s_tanh_ur(c, d, xt, m2):
                d['h_g'] = d['tp'].tile([NGRU, 2 * BC], bf16, tag="h_g",
                                        name=f"hg{c}")
                d['ur'] = nc.scalar.activation(d['h_g'][:, :], d['g1'][:, :],
                                               ACT.Tanh)

            def s_sig(c, d, xt, m2):
                d['vr'] = d['tp'].tile([2 * LAT, 2 * BC], bf16, tag="vr",
                                       name=f"vr{c}")
                d['sig'] = nc.scalar.activation(d['vr'][:, :], d['g2'][:, :],
                                                ACT.Sigmoid)

            def s_tanh_ns(c, d, xt, m2):
                d['h_n'] = d['tp'].tile([NGRU, BC], bf16, tag="h_n",
                                        name=f"hn{c}")
                nc.scalar.activation(d['h_n'][:, :], d['n1'][:, :], ACT.Tanh,
                                     bias=bns1[:, :] if use_bias else 0.0)

            # -- DVE stages --
            def s_yode(c, d, xt, m2):
                nc.vector.scalar_tensor_tensor(
                    d['Sc'][0:LAT, :], d['ps_yo'][:, :], float(dts[d['t']]),
                    d['Sc'][0:LAT, :],
                    op0=ALU.mult, op1=ALU.add)

            def s_gm(c, d, xt, m2):
                d['g'] = d['tp'].tile([2 * LAT, BC], bf16, tag="g",
                                      name=f"g{c}")
                nc.vector.tensor_mul(d['g'][:, :], m2[:, d['cs']],
                                     d['vr'][:, 0:BC])

            def s_ryc(c, d, xt, m2):
                d['ryc'] = d['tp'].tile([2 * LAT, BC], bf16, tag="ryc",
                                        name=f"ryc{c}")
                nc.vector.tensor_mul(d['ryc'][:, :], d['vr'][:, BC:2 * BC],
                                     d['Sc'][:, :])

            def s_dift(c, d, xt, m2):
                # full diff: top = ns_y - y_ode; bottom = ns_s - s (one op)
                d['dt'] = d['tp'].tile([2 * LAT, BC], bf16, tag="dt",
                                       name=f"dt{c}")
                nc.vector.tensor_sub(d['dt'][:, :], d['n2'][:, :],
                                     d['Sc'][:, :])

            def s_q2(c, d, xt, m2):
                # q2 = -ns_s - s; then |ns_s| - s = max(ns_s - s, q2)
                d['ab'] = d['tp'].tile([2 * LAT, BC], bf16, tag="ab",
                                       name=f"ab{c}")
                nc.vector.scalar_tensor_tensor(
                    d['ab'][LAT:2 * LAT, :], d['n2'][LAT:2 * LAT, :], -1.0,
                    d['Sc'][LAT:2 * LAT, :],
                    op0=ALU.mult, op1=ALU.subtract)

            def s_maxb(c, d, xt, m2):
                nc.vector.tensor_max(d['dt'][LAT:2 * LAT, :],
                                     d['dt'][LAT:2 * LAT, :],
                                     d['ab'][LAT:2 * LAT, :])

            def s_gtq(c, d, xt, m2):
                d['gq'] = d['tp'].tile([2 * LAT, BC], bf16, tag="gq",
                                      name=f"gq{c}")
                nc.vector.tensor_mul(d['gq'][:, :], d['g'][:, :],
                                     d['dt'][:, :])

            def s_add(c, d, xt, m2):
                nc.vector.tensor_add(d['Sn'][:, :], d['Sc'][:, :],
                                     d['gq'][:, :])

            from concourse.tile import add_dep_helper

            # NOTE: a matmul's start=True clears has_written for its whole
            # PSUM bank, so the u group (xu..su) must fully close before the
            # r group (xr..sr) opens in the shared g1 bank.
            stages = [s_ode1, s_xu, s_tanh_ode, s_ode2, s_xn, s_yode,
                      s_su, s_xr, s_sr, s_tanh_ur, s_g2u, s_g2r, s_sig,
                      s_gm, s_ryc, s_ns1, s_tanh_ns, s_ns2, s_dift,
                      s_q2, s_maxb, s_gtq, s_add]
            NS = len(stages)
            SIG_IDX = stages.index(s_tanh_ur)
            OFF = NS // 2
            total = T * NS
            ctx = [None, None]
            xts = {}
            last_sig = [None, None]
            for n in range(total + OFF):
                if n < total:
                    t, k = divmod(n, NS)
                    if k == 0:
                        xt = xpool.tile([INP, B], bf16, tag="xt",
                                        name=f"xt{t % 4}")
                        nc.sync.dma_start(xt[:, :], xT_d[t])
                        m2 = mpool.tile([INP, B], bf16, tag="m2",
                                        name=f"m2_{t % 4}")
                        nc.gpsimd.dma_start(m2[0:NDATA, :], xt[NDATA:INP, :])
                        nc.gpsimd.dma_start(m2[NDATA:INP, :], xt[NDATA:INP, :])
                        xts[t] = (xt, m2)
                        ctx[0] = new_ctx(0, t)
                    stages[k](0, ctx[0], *xts[t])
                    if k == 0 and last_sig[1] is not None:
                        add_dep_helper(ctx[0]['ode1'].ins, last_sig[1].ins,
                                       sync=False, reason="anti-phase c0<-c1")
                    if k == SIG_IDX:
                        last_sig[0] = ctx[0].get('ur')
                m = n - OFF
                if m >= 0:
                    t, k = divmod(m, NS)
                    if k == 0:
                        ctx[1] = new_ctx(1, t)
                    stages[k](1, ctx[1], *xts[t])
                    if k == 0 and last_sig[0] is not None:
                        add_dep_helper(ctx[1]['ode1'].ins, last_sig[0].ins,
                                       sync=False, reason="anti-phase c1<-c0")
                    if k == SIG_IDX:
                        last_sig[1] = ctx[1].get('ur')
                    if k == NS - 1:
                        xts.pop(t)

            if globals().get("_DBG_TAPS"):
                d0 = ctx[0]
                taps = {"xt": d0['xt'], "m2": d0['m2'], "h_ode": d0['h_ode'],
                        "h_g": d0['h_g'], "vr": d0['vr'], "g": d0['g'],
                        "ryc": d0['ryc'], "h_n": d0['h_n'], "dt": d0['dt'],
                        "ab": d0['ab'], "gq": d0['gq'], "Sn": d0['Sn']}
                for nm, tl in taps.items():
                    shp = list(tl.shape)
                    dd = nc.dram_tensor(f"dbg_{nm}", shp, bf16,
                                        kind="ExternalOutput")
                    nc.sync.dma_start(dd[:, :], tl[:, :])

            # ---- final transform z0 = mlp2([y; s]) ----
            for c in range(CH):
                cs = slice(c * BC, (c + 1) * BC)
                Sf = S[c][T % 2]
                pt1 = psA[c].tile([TZ, BC], f32, tag="psA")
                nc.tensor.matmul(pt1[:, :], wtz1[:, :], Sf[:, :],
                                 start=True, stop=True)
                h_t = tpool[c].tile([TZ, BC], bf16, tag="h_t")
                nc.scalar.activation(h_t[:, :], pt1[:, :], ACT.Tanh,
                                     bias=btz1[:, :] if use_bias else 0.0)
                pt2 = psB[c].tile([2 * LAT, BC], f32, tag="psB")
                nc.tensor.matmul(pt2[:, :], wtz2[:, :], h_t[:, :],
                                 start=True, stop=True)
                zo = tpool[c].tile([2 * LAT, BC], f32, tag="zo")
                nc.scalar.activation(zo[0:LAT, :], pt2[0:LAT, :], ACT.Copy,
                                     bias=btz2t[:, :] if use_bias else 0.0)
                nc.scalar.activation(zo[LAT:2 * LAT, :], pt2[LAT:2 * LAT, :],
                                     ACT.Abs,
                                     bias=btz2b[LAT:2 * LAT, :] if use_bias else 0.0)
                nc.sync.dma_start(zout_d[:, cs], zo[:, :])

    nc.compile()
    return nc


def _prep(inputs):
    import ml_dtypes
    BF = ml_dtypes.bfloat16

    g = lambda k: np.ascontiguousarray(np.asarray(inputs[k], dtype=np.float32))
    data = g("data")
    tps = g("tps")
    W = {k: g(k) for k in (
        "ug_w1", "ug_b1", "ug_w2", "ug_b2", "rg_w1", "rg_b1", "rg_w2", "rg_b2",
        "ns_w1", "ns_b1", "ns_w2", "ns_b2", "ode_w1", "ode_b1", "ode_w2",
        "ode_b2", "tz_w1", "tz_b1", "tz_w2", "tz_b2")}

    rev = tps[::-1]
    dts = np.concatenate([np.full((1,), -0.01, np.float32),
                          rev[1:] - rev[:-1]]).astype(np.float32)
    dts = tuple(float(d) for d in dts.tolist())

    use_bias = any(float(np.abs(W[k]).max()) != 0.0 for k in W if "_b" in k)

    # time-reverse + transpose: [T, INP, N_TRAJ], contiguous, bf16
    xT_full = np.ascontiguousarray(
        data[:, ::-1, :].transpose(1, 2, 0).astype(BF))

    bfc = lambda v: np.ascontiguousarray(v.astype(BF))
    common = {
        "wug1": bfc(W["ug_w1"]),
        "wrg1": bfc(W["rg_w1"]),
        "wns1": bfc(W["ns_w1"]),
        "wug2nd": bfc(-np.concatenate([W["ug_w2"], W["ug_w2"]], axis=1)),
        "wrg2d": bfc(np.concatenate([W["rg_w2"], W["rg_w2"]], axis=1)),
        "wns2": bfc(W["ns_w2"]),
        "wode1": bfc(W["ode_w1"]),
        "wode2": bfc(W["ode_w2"]),
        "wtz1": bfc(W["tz_w1"]),
        "wtz2": bfc(W["tz_w2"]),
        "zeros0": np.zeros((2 * LAT, B), BF),
    }
    if use_bias:
        col = lambda v: np.ascontiguousarray(v.reshape(-1, 1).astype(np.float32))
        row = lambda v: np.ascontiguousarray(v.reshape(1, -1).astype(BF))
        common.update({
            "bode1": col(W["ode_b1"]),
            "bns1": col(W["ns_b1"]),
            "btz1": col(W["tz_b1"]),
            "btz2t": col(W["tz_b2"][:LAT]),
            "btz2b": col(W["tz_b2"][LAT:]),
            "bug1r": row(W["ug_b1"]),
            "brg1r": row(W["rg_b1"]),
            "bug2ndr": row(-np.concatenate([W["ug_b2"], W["ug_b2"]])),
            "brg2dr": row(np.concatenate([W["rg_b2"], W["rg_b2"]])),
            "bns2r": row(W["ns_b2"]),
            "bode2r": row(W["ode_b2"]),
            "ones1": np.ones((1, BC), BF),
        })

    in_maps = []
    for c in range(NCORES):
        m = dict(common)
        m["xT"] = np.ascontiguousarray(xT_full[:, :, c * B:(c + 1) * B])
        in_maps.append(m)
    return in_maps, dts, use_bias


def _ensure_ntff_hook():
    """Install a stub antenv.axon_hooks if absent so tracing can't crash."""
    import types as _types
    if "antenv.axon_hooks" in sys.modules:
        return
    hook = None
    try:
        from trn_agent_boot.trn_boot import _ntff_profile_via_ctypes
        hook = _ntff_profile_via_ctypes("/opt/axon/libaxon_pjrt.so")
    except Exception:
        hook = None
    try:
        import antenv
        mod = _types.ModuleType("antenv.axon_hooks")
        mod.get_axon_ntff_profile_hook = lambda: hook
        mod.set_axon_ntff_profile_hook = lambda h: None
        sys.modules["antenv.axon_hooks"] = mod
        antenv.axon_hooks = mod
    except Exception:
        pass


def _run(inputs, trace=False, trace_kwargs=None):
    _ensure_ntff_hook()
    from concourse.bass_utils import run_bass_kernel_spmd

    in_maps, dts, use_bias = _prep(inputs)
    key = (dts, use_bias)
    if key not in _cache:
        _cache[key] = _build(dts, use_bias)
    nc = _cache[key]

    res = run_bass_kernel_spmd(nc, in_maps, list(range(NCORES)),
                               trace=trace, **(trace_kwargs or {}))
    mu = np.empty((N_TRAJ, LAT), np.float32)
    std = np.empty((N_TRAJ, LAT), np.float32)
    for c in range(NCORES):
        z = res.results[c]["zout"]
        mu[c * B:(c + 1) * B] = z[0:LAT].T
        std[c * B:(c + 1) * B] = z[LAT:2 * LAT].T
    return (mu[None], std[None]), res


def kernel(**inputs):
    out, _ = _run(inputs, trace=False)
    return out
